# revision 43
# baseline (speedup 1.0000x reference)
"""Distributed attention kernel for 8 TRN2 NeuronCores.

Sharding: tensor-parallel over heads (2 heads/core, Megatron column split of
w_qkv), attention computed per-core for its heads over all batches, then a
per-batch AllToAll redistributes the (transposed) attention output so each
core runs the output projection for 1/8 of the tokens (256 per batch) against
the full w_proj.

Layout: everything is kept transposed (d on partitions) so that
  - scores come out as S^T (keys on partitions, queries on free axis),
  - softmax needs no max subtraction (logits ~ N(0,1)),
  - the two heads run as row/col-tiled concurrent matmul pairs using the full
    128-wide PE array,
  - the projection consumes the transposed attention output directly.
Compute dtype is bf16 with f32 PSUM accumulation.

The build is software-pipelined: QKV for batch b+1 and the projection for
batch b-1 are interleaved into attention(b)'s inner loop as filler units so
the TensorEngine never idles long enough for the HAM clock gate to throttle
it. Softmax denominators are accumulated on the VectorEngine (two bf16
accumulators per strip), reduced across partitions by a ones-matmul, batched
through DRAM so one DVE reciprocal serves a whole batch, and broadcast back
by a partition-stride-0 DMA.
"""

import os
import sys

import numpy as np

for _p in ("/opt/trn_rl_repo", os.path.expanduser("~/.axon_site/_ro/trn_rl_repo")):
    if os.path.isdir(_p) and _p not in sys.path:
        sys.path.insert(0, _p)

import ml_dtypes  # noqa: E402

import concourse.bass as bass  # noqa: E402
from concourse import bacc, bass_isa, mybir  # noqa: E402
import concourse.tile as tile  # noqa: E402
from concourse.bass_utils import run_bass_kernel_spmd  # noqa: E402

B, N, DIM, H = 4, 2048, 1024, 16
HD = DIM // H            # 64 head dim
NCORES = 8
HPC = H // NCORES        # 2 heads per core
HC = HPC * HD            # 128 head-cols per core
T = B * N                # 8192 tokens
QT_TOK = N // NCORES     # 256 tokens per core per batch
SCALE = HD ** -0.5

BF16 = mybir.dt.bfloat16
F32 = mybir.dt.float32
EXP = mybir.ActivationFunctionType.Exp

LAST_RESULTS = None  # BassKernelResults of the most recent run (for test.py)


def _build():
    nc = bacc.Bacc(num_devices=NCORES)

    x_t = nc.declare_dram_parameter("x_t", [DIM, T], BF16, isOutput=False)
    w_c = nc.declare_dram_parameter("w_c", [DIM, 3 * HC], BF16, isOutput=False)
    w_p = nc.declare_dram_parameter("w_p", [DIM, DIM], BF16, isOutput=False)
    b_p = nc.declare_dram_parameter("b_p", [DIM], F32, isOutput=False)
    # columns: batch-major quarters of 256 tokens
    out_t = nc.declare_dram_parameter("out_t", [DIM, B * QT_TOK], F32, isOutput=True)

    with tile.TileContext(nc) as tc:
        with (
            tc.tile_pool(name="persist", bufs=1) as persist,
            tc.tile_pool(name="xin", bufs=3) as xin,
            tc.tile_pool(name="work", bufs=3) as work,
            tc.tile_pool(name="ps_mm", bufs=2, space="PSUM") as ps_mm,
            tc.tile_pool(name="ps_s", bufs=2, space="PSUM") as ps_s,
            tc.tile_pool(name="ps_o", bufs=2, space="PSUM") as ps_o,
            tc.tile_pool(name="dram", bufs=1, space="DRAM") as dram,
        ):
            # ---- persistent SBUF tensors ----
            wqkv_sb = persist.tile([128, 8, 3 * HC], BF16)
            wproj_sb = persist.tile([128, 8, DIM], BF16)
            bias_sb = persist.tile([128, 8], F32)
            ones_sb = persist.tile([128, 1], BF16)
            QT = persist.tile([128, T], BF16)
            KT = persist.tile([128, T], BF16)
            Vp = persist.tile([128, B, 16, HPC, HD], BF16)
            attnT = persist.tile([128, T], BF16)   # rows = h*64 + d

            # ---- DRAM staging ----
            den_d = dram.tile([B, 2 * 4, 512], BF16)    # idx = qi*2 + h
            rden_d = dram.tile([B, 2 * 4, 512], BF16)
            ag_in = dram.tile([B, NCORES * HC, QT_TOK], BF16)
            ag_out = dram.tile([B, NCORES * HC, QT_TOK], BF16)
            ag_in3 = dram.tile([2, NCORES * HC, 128], BF16)
            ag_out3 = dram.tile([2, NCORES * HC, 128], BF16)

            for k in range(8):
                nc.sync.dma_start(wqkv_sb[:, k, :], w_c[k * 128:(k + 1) * 128, :])
            nc.vector.memset(ones_sb, 1.0)

            # ---- phase builders ----
            def qkv_chunk_units(tq, kv_first=False):
                """1024-token QKV chunk as a list of filler closures."""
                st = {}

                def u_dma():
                    xt = xin.tile([128, 8, 1024], BF16, tag="xt", name=f"xt{tq}")
                    for nh in range(2):
                        for k in range(8):
                            c0 = tq * 1024 + nh * 512
                            nc.sync.dma_start(
                                xt[:, k, nh * 512:(nh + 1) * 512],
                                x_t[k * 128:(k + 1) * 128, c0:c0 + 512],
                            )
                    st["xt"] = xt

                def mk_qk(m, nh, half):
                    def u():
                        xt = st["xt"]
                        if half == 0:
                            st[(m, nh)] = ps_mm.tile(
                                [128, 512], F32, tag="mm", name=f"pqk{tq}{m}{nh}"
                            )
                        pmm = st[(m, nh)]
                        for k in range(4 * half, 4 * half + 4):
                            nc.tensor.matmul(
                                pmm,
                                wqkv_sb[:, k, m * 128:(m + 1) * 128],
                                xt[:, k, nh * 512:(nh + 1) * 512],
                                start=(k == 0),
                                stop=(k == 7),
                            )
                        if half == 1:
                            dst = QT if m == 0 else KT
                            nc.vector.tensor_copy(
                                dst[:, tq * 1024 + nh * 512:
                                    tq * 1024 + (nh + 1) * 512],
                                pmm,
                            )
                    return u

                def mk_v(st_idx):
                    def u():
                        xt = st["xt"]
                        pv = ps_mm.tile([128, 128], F32, tag="mm", name=f"pv{tq}{st_idx}")
                        for k in range(8):
                            nc.tensor.matmul(
                                pv,
                                xt[:, k, st_idx * 128:(st_idx + 1) * 128],
                                wqkv_sb[:, k, 2 * HC:3 * HC],
                                start=(k == 0),
                                stop=(k == 7),
                            )
                        gt = tq * 8 + st_idx
                        b, lt = gt // 16, gt % 16
                        nc.vector.tensor_copy(Vp[:, b, lt, :, :], pv)
                    return u

                if kv_first:
                    # K and V as early as possible (attention consumes them
                    # incrementally along the kj axis); Q strips last
                    units = [u_dma]
                    for nh in range(2):
                        units += [mk_qk(1, nh, 0), mk_qk(1, nh, 1)]
                    for st_idx in range(8):
                        units.append(mk_v(st_idx))
                    for nh in range(2):
                        units += [mk_qk(0, nh, 0), mk_qk(0, nh, 1)]
                else:
                    units = [u_dma]
                    for m in range(2):
                        for nh in range(2):
                            units.append(mk_qk(m, nh, 0))
                            units.append(mk_qk(m, nh, 1))
                    for st_idx in range(8):
                        units.append(mk_v(st_idx))
                return units

            def proj_quarter_units(b):
                """Projection of this core's 256 batch-b tokens as fillers."""
                st = {}

                def u_dma():
                    agT = work.tile([128, 8, QT_TOK], BF16, tag="agT", name=f"agT{b}")
                    for r in range(NCORES):
                        nc.sync.dma_start(
                            agT[:, r, :], ag_out[b, r * HC:(r + 1) * HC, :]
                        )
                    st["agT"] = agT

                def mk_od(od, half):
                    def u():
                        agT = st["agT"]
                        if half == 0:
                            st[od] = ps_mm.tile(
                                [128, QT_TOK], F32, tag="mm", name=f"pp{b}{od}"
                            )
                        pp = st[od]
                        for r in range(4 * half, 4 * half + 4):
                            nc.tensor.matmul(
                                pp,
                                wproj_sb[:, r, od * 128:(od + 1) * 128],
                                agT[:, r, :],
                                start=(r == 0),
                                stop=(r == 7),
                            )
                        if half == 1:
                            ob = work.tile([128, QT_TOK], F32, tag="ob",
                                           name=f"ob{b}{od}")
                            nc.vector.tensor_scalar_add(
                                ob, pp, bias_sb[:, od:od + 1]
                            )
                            nc.sync.dma_start(
                                out_t[od * 128:(od + 1) * 128,
                                      b * QT_TOK:(b + 1) * QT_TOK],
                                ob,
                            )
                    return u

                return [u_dma] + [mk_od(od, hf) for od in range(8) for hf in (0, 1)]

            def norm_half(b, half):
                """Reciprocal + broadcast-multiply for qi strips 2h, 2h+1."""
                t0 = b * N
                den_sb = work.tile([4, 512], BF16, tag="den", name=f"den{b}{half}")
                nc.sync.dma_start(den_sb, den_d[b, 4 * half:4 * half + 4, :])
                rden_f = work.tile([4, 512], F32, tag="rdenf", name=f"rdf{b}{half}")
                nc.vector.reciprocal(rden_f, den_sb)
                rden_b = work.tile([4, 512], BF16, tag="rdenb", name=f"rdb{b}{half}")
                nc.vector.tensor_copy(rden_b, rden_f)
                nc.sync.dma_start(rden_d[b, 4 * half:4 * half + 4, :], rden_b)
                for qi in (2 * half, 2 * half + 1):
                    q0 = t0 + qi * 512
                    bc = work.tile([128, 512], BF16, tag="bc")
                    for h in range(HPC):
                        src = rden_d[b, qi * 2 + h, :]
                        bcast = bass.AP(tensor=src.tensor, offset=src.offset,
                                        ap=[[0, HD], [1, 512]])
                        nc.sync.dma_start(bc[h * HD:(h + 1) * HD, :], bcast)
                    nc.vector.tensor_mul(
                        attnT[:, q0:q0 + 512], attnT[:, q0:q0 + 512], bc
                    )

            def a2a3_half(half):
                base = 3 * N + half * 1024
                for j in range(NCORES):
                    nc.sync.dma_start(
                        ag_in3[half, j * HC:(j + 1) * HC, :],
                        attnT[:, base + j * 128:base + (j + 1) * 128],
                    )
                nc.gpsimd.collective_compute(
                    "AllToAll", mybir.AluOpType.bypass,
                    replica_groups=[list(range(NCORES))],
                    ins=[ag_in3[half]], outs=[ag_out3[half]],
                )

            def proj_half_units(half):
                st = {}

                def u_dma():
                    agT = work.tile([128, 8, 128], BF16, tag="agT3",
                                    name=f"agT3{half}")
                    for r in range(NCORES):
                        nc.sync.dma_start(
                            agT[:, r, :], ag_out3[half, r * HC:(r + 1) * HC, :]
                        )
                    st["agT"] = agT

                def mk_od(od):
                    def u():
                        agT = st["agT"]
                        pp = ps_mm.tile([128, 128], F32, tag="mm",
                                        name=f"p3{half}{od}")
                        for r in range(8):
                            nc.tensor.matmul(
                                pp, wproj_sb[:, r, od * 128:(od + 1) * 128],
                                agT[:, r, :], start=(r == 0), stop=(r == 7),
                            )
                        ob = work.tile([128, 128], F32, tag="ob",
                                       name=f"ob3{half}{od}")
                        nc.vector.tensor_scalar_add(ob, pp, bias_sb[:, od:od + 1])
                        nc.sync.dma_start(
                            out_t[od * 128:(od + 1) * 128,
                                  3 * QT_TOK + half * 128:
                                  3 * QT_TOK + half * 128 + 128],
                            ob,
                        )
                    return u

                return [u_dma] + [mk_od(od) for od in range(8)]

            # ---- minimal batch-0 prologue: x chunk 0, Q/K for kj 0-3, V 0-1
            ch0 = qkv_chunk_units(0)
            for i in (0, 1, 2, 5, 6, 9, 10):
                ch0[i]()
            ch0_rest = [ch0[i] for i in (7, 8, 11, 12, 13, 14, 15, 16, 3, 4)]
            # weight loads for the projection go on the idle gpsimd DMA queue
            # so they don't clog the sync queue ahead of x/attention traffic
            for k in range(8):
                nc.sync.dma_start(wproj_sb[:, k, :], w_p[k * 128:(k + 1) * 128, :])
                nc.sync.dma_start(bias_sb[:, k:k + 1], b_p[k * 128:(k + 1) * 128])

            # ---- pipelined main loop over batches ----
            for b in range(B):
                t0 = b * N
                # each batch's second chunk drains K/V-first inside its own
                # attention (consumed incrementally along kj); the next
                # batch's first chunk follows; proj(b-1) last.
                if b == 0:
                    fillers = (ch0_rest + qkv_chunk_units(1, kv_first=True)
                               + qkv_chunk_units(2))
                else:
                    fillers = qkv_chunk_units(2 * b + 1, kv_first=True)
                    if b + 1 < B:
                        fillers += qkv_chunk_units(2 * (b + 1))
                    fillers += proj_quarter_units(b - 1)
                # qkv fillers are paced from iteration 0; proj fillers (which
                # wait on the previous batch's AllToAll) only from PROJ_AT on.
                n_qkv = len(fillers) - (17 if b >= 1 else 0)
                PROJ_AT = 20
                fillers.reverse()  # pop() from the end = original order
                n_fill = len(fillers)
                popped = 0

                # flat list of (qi, kj) steps; S^T pairs are emitted one step
                # ahead so the ACT-feeding matmul is never queued behind the
                # eS-gated V matmuls or filler work on the in-order PE queue
                steps = [(qi, kj) for qi in range(4) for kj in range(16)]
                pS_t = {}
                po_t = {}
                acc_t = {}

                def emit_S(qi, kj):
                    q0 = t0 + qi * 512
                    k0 = t0 + kj * 128
                    pS = ps_s.tile([128, 2, 512], F32, tag="s",
                                   name=f"pS{b}_{qi}_{kj}")
                    for h in range(HPC):
                        hs = h * HD
                        nc.tensor.matmul(
                            pS[:, h, :],
                            KT[hs:hs + HD, k0:k0 + 128],
                            QT[hs:hs + HD, q0:q0 + 512],
                            start=True,
                            stop=True,
                        )
                    pS_t[(qi, kj)] = pS

                emit_S(0, 0)
                for it, (qi, kj) in enumerate(steps):
                    q0 = t0 + qi * 512
                    if kj == 0:
                        po_t[qi] = ps_o.tile([128, 512], F32, tag="vo",
                                             name=f"po{b}_{qi}")
                        acc_t[qi] = [
                            work.tile([128, 2, 512], BF16, tag=f"acc{a}",
                                      name=f"acc{a}_{b}_{qi}")
                            for a in range(2)
                        ]
                    # floor of 2/iter until the 13 K/V-critical units of the
                    # own-batch chunk are emitted, else the V matmuls of kj
                    # 8-15 would be traced before their producer (trace-order
                    # deps would read garbage)
                    target = max((it + 1) * n_fill // 56,
                                 min(13, 2 * (it + 1)))
                    if b == 0 and it < 16:
                        target = 2 * (it + 1)
                    quota = popped < n_qkv or it >= PROJ_AT
                    due = max(0, target - popped) if quota else 0
                    # sandwich the filler work around the two sem-gated
                    # instructions (S waiting its PSUM slot, V waiting eS) so
                    # the in-order PE queue never idles at a blocked head
                    for _ in range(due // 2):
                        if fillers:
                            fillers.pop()()
                            popped += 1
                    if it + 1 < len(steps):
                        emit_S(*steps[it + 1])
                    pS = pS_t.pop((qi, kj))
                    eS = work.tile([128, 2, 512], BF16, tag="es", bufs=4)
                    nc.scalar.activation(eS, pS, EXP, scale=SCALE)
                    for _ in range(due - due // 2):
                        if fillers:
                            fillers.pop()()
                            popped += 1
                    po, acc = po_t[qi], acc_t[qi]
                    for h in range(HPC):
                        nc.tensor.matmul(
                            po[h * HD:(h + 1) * HD, :],
                            Vp[:, b, kj, h, :],
                            eS[:, h, :],
                            start=(kj == 0),
                            stop=(kj == 15),
                        )
                    a = kj // 8
                    if kj % 8 == 0:
                        nc.vector.tensor_copy(acc[a], eS)
                    else:
                        nc.vector.tensor_add(acc[a], acc[a], eS)
                    if kj == 15:
                        # stage numerators (unnormalized, one copy, both heads)
                        nc.vector.tensor_copy(attnT[:, q0:q0 + 512], po)
                        # denominators: partition-reduce the accumulators
                        nc.vector.tensor_add(acc[0], acc[0], acc[1])
                        for h in range(HPC):
                            idx = qi * 2 + h
                            pden = ps_mm.tile([1, 512], F32, tag="mm",
                                              name=f"pden{b}{idx}")
                            nc.tensor.matmul(pden, ones_sb[:, 0:1],
                                             acc[0][:, h, :],
                                             start=True, stop=True)
                            dstage = work.tile([1, 512], BF16, tag="dst")
                            nc.vector.tensor_copy(dstage, pden)
                            nc.sync.dma_start(den_d[b, idx, :], dstage)
                        if False and b == B - 1 and qi == 1:
                            # first half of the last batch: redistribute and
                            # project while qi strips 2-3 are still computing
                            norm_half(b, 0)
                            a2a3_half(0)
                            new = proj_half_units(0)
                            fillers[:0] = list(reversed(new))
                            n_fill += len(new)
                while fillers:
                    fillers.pop()()

                if True:
                    norm_half(b, 0)
                    norm_half(b, 1)
                    # ---- AllToAll for batch b ----
                    for j in range(NCORES):
                        nc.sync.dma_start(
                            ag_in[b, j * HC:(j + 1) * HC, :],
                            attnT[:, t0 + j * QT_TOK:t0 + (j + 1) * QT_TOK],
                        )
                    nc.gpsimd.collective_compute(
                        "AllToAll",
                        mybir.AluOpType.bypass,
                        replica_groups=[list(range(NCORES))],
                        ins=[ag_in[b]],
                        outs=[ag_out[b]],
                    )

            # ---- projection for the last batch ----
            for u in proj_quarter_units(B - 1):
                u()

    nc.finalize()
    return nc


def kernel(x, w_qkv, w_proj, b_proj):
    global LAST_RESULTS
    bf16 = ml_dtypes.bfloat16

    x_t = np.ascontiguousarray(x.reshape(T, DIM).T.astype(bf16))  # [DIM, T]
    w_p = np.ascontiguousarray(w_proj.astype(bf16))
    b_p = np.ascontiguousarray(b_proj.astype(np.float32))

    in_maps = []
    for c in range(NCORES):
        w_c = np.concatenate(
            [
                w_qkv[:, HC * c:HC * (c + 1)],
                w_qkv[:, DIM + HC * c:DIM + HC * (c + 1)],
                w_qkv[:, 2 * DIM + HC * c:2 * DIM + HC * (c + 1)],
            ],
            axis=1,
        ).astype(bf16)
        in_maps.append(
            {"x_t": x_t, "w_c": np.ascontiguousarray(w_c), "w_p": w_p, "b_p": b_p}
        )

    nc = _build()
    LAST_RESULTS = run_bass_kernel_spmd(
        nc, in_maps, core_ids=list(range(NCORES)),
        trace=bool(os.environ.get("KERNEL_TRACE")),
    )

    # core c's out_t columns are batch-major quarters of its 256-token slices
    out_T = np.empty((DIM, T), dtype=np.float32)
    for c in range(NCORES):
        res = np.asarray(LAST_RESULTS.results[c]["out_t"], dtype=np.float32)
        for b in range(B):
            out_T[:, b * N + c * QT_TOK:b * N + (c + 1) * QT_TOK] = (
                res[:, b * QT_TOK:(b + 1) * QT_TOK]
            )
    return np.ascontiguousarray(out_T.T).reshape(B, N, DIM).astype(np.float32)
